# revision 19
# baseline (speedup 1.0000x reference)
"""Trainium2 Bass kernel for GQA multi-head attention (nn_MultiHeadAttention).

Problem (hardcoded): B=2, S=2048, DIM=2048, H=32 q-heads, KVH=8 kv-heads,
HD=64, rotate-half RoPE theta=10000, causal, out-proj + bias. All fp32 I/O.

Sharding over 8 NeuronCores (SPMD, one program):
  core c -> batch b=c//4, head-group g=c%4 (q heads 8g..8g+7 = kv heads 2g,2g+1,
  keeping each kv head's 4 q heads together). Each core computes qkv projection
  for its head group, RoPE, causal attention with the softmax denominator
  folded into the AV matmul via an appended ones-column on V, and a partial
  out-projection over its 512 head dims. The 4 cores of a batch ReduceScatter
  (bf16) the partial projections in 512x512 column-quarter pieces (16 total),
  pipelined behind compute; each core returns 4x128 rows of the final output.
  Host adds the bias and concatenates.

Numerics: all matmuls in bf16 with fp32 PSUM accumulation; x and all weights
are cast to bf16 on the HOST (no device-side staging/casts); exp on ScalarE in
fp32 from PSUM with the 1/sqrt(HD) scale folded into the activation's free
affine; no max-subtraction (scores are O(5) for these inputs).

DMA queues: weights on the Scalar HWDGE queue, x tiles + kdup/ysb/y writes on
the GpSimd software DGE, collective staging + small constants on Sync - the
ReduceScatter staging copy can head-of-line block its queue, so nothing
latency-critical shares the Sync queue with it.
"""
from collections import deque as _deque

import numpy as np
import ml_dtypes

import concourse.bass as bass
import concourse.bacc as bacc
import concourse.tile as tile
import concourse.mybir as mybir
from concourse.bass_utils import run_bass_kernel_spmd

BF16 = mybir.dt.bfloat16
F32 = mybir.dt.float32
FP8 = mybir.dt.float8e4
AF = mybir.ActivationFunctionType
EXP_SHIFT = -2.0     # exp(scale*s - 2): max score*scale is 5.59 -> e^3.59=36
                     # fits fp8e4 (max 240); the uniform e^-2 cancels in the
                     # softmax normalization (ones-column denominator shares it)

DIM, H, KVH, HD, B, S = 2048, 32, 8, 64, 2, 2048
NCORES = 8
SCALE = float(1.0 / np.sqrt(HD))
KT = DIM // 128          # 16 contraction tiles
NQC = 4                  # 512-wide sequence chunks
THETA = 10000.0

_CACHED_NC = None


def _pin_act_tables():
    """Point walrus at a table root containing only natural_log_exp_and_others.

    The kernel's ScalarE functions (Exp, Ln, Copy) all live in that one set,
    but walrus's per-function set choice otherwise thrashes between
    exp_and_others and natural_log (65 ACT_TABLE_LOADs = 83us measured).
    """
    import os
    import tempfile
    import json as _json

    if os.environ.get("BASS_ACT_ROOT_JSON_PATH"):
        return
    import neuronxcc

    src_dir = os.path.join(os.path.dirname(neuronxcc.__file__),
                           "pwp", "pwp_bin_trainium")
    src_json = os.path.join(src_dir, "act_info.json")
    if not os.path.exists(src_json):
        return
    with open(src_json) as f:
        info = _json.load(f)
    keep = [s for s in info["act_func_sets"]
            if s.get("name") == "natural_log_exp_and_others"]
    if not keep:
        return
    info["act_func_sets"] = keep
    dst = tempfile.mkdtemp(prefix="act_pinned_")
    for fn in os.listdir(src_dir):
        if fn != "act_info.json":
            os.symlink(os.path.join(src_dir, fn), os.path.join(dst, fn))
    with open(os.path.join(dst, "act_info.json"), "w") as f:
        _json.dump(info, f)
    os.environ["BASS_ACT_ROOT_JSON_PATH"] = os.path.join(dst, "act_info.json")

    import concourse.hw_specs as hw_specs
    orig = hw_specs.get_activation_tables

    def pinned(arch):
        t = orig(arch)
        return {"natural_log_exp_and_others": t["natural_log_exp_and_others"]}

    hw_specs.get_activation_tables = pinned
    bacc.get_activation_tables = pinned


def build_nc():
    """Build (and cache) the single SPMD Bass program."""
    global _CACHED_NC
    if _CACHED_NC is not None:
        return _CACHED_NC

    _pin_act_tables()
    nc = bacc.Bacc("TRN2", target_bir_lowering=False, debug=False,
                   num_devices=NCORES)

    xt_d = nc.dram_tensor("xt", [DIM, S], BF16, kind="ExternalInput")
    wq_d = nc.dram_tensor("wq", [DIM, 512], BF16, kind="ExternalInput")
    wk_d = nc.dram_tensor("wk", [DIM, 128], BF16, kind="ExternalInput")
    wv_d = nc.dram_tensor("wv", [DIM, 128], BF16, kind="ExternalInput")
    wp_d = nc.dram_tensor("wp", [512, DIM], BF16, kind="ExternalInput")
    cos_d = nc.dram_tensor("cost", [128, S], F32, kind="ExternalInput")
    sin_d = nc.dram_tensor("sint", [128, S], F32, kind="ExternalInput")
    r2t_d = nc.dram_tensor("r2t", [128, 128], BF16, kind="ExternalInput")
    mask_d = nc.dram_tensor("maskt", [128, 2048], BF16, kind="ExternalInput")
    y_d = nc.dram_tensor("y", [512, DIM], BF16, kind="ExternalOutput")
    dbg_out_d = nc.dram_tensor("dbg_out", [128, S], BF16, kind="ExternalOutput")
    dbg_po_d = nc.dram_tensor("dbg_po", [65, 512], F32, kind="ExternalOutput")
    dbg_rf_d = nc.dram_tensor("dbg_rf", [1, 512], F32, kind="ExternalOutput")
    dbg_ex_d = nc.dram_tensor("dbg_ex", [128, 1024], BF16, kind="ExternalOutput")

    groups = [[0, 1, 2, 3], [4, 5, 6, 7]]

    with tile.TileContext(nc) as tc:
        with (
            tc.tile_pool(name="sb", bufs=1) as sb,
            tc.tile_pool(name="ps", bufs=1, space="PSUM") as ps,
            tc.tile_pool(name="dr", bufs=1, space="DRAM") as dr,
        ):
            # ---- constants / persistent tiles (Sync queue: small, early) ----
            ones64 = sb.tile([1, 64], BF16, tag="c0", bufs=1)
            nc.vector.memset(ones64[:], 1.0)
            cos_sb = sb.tile([128, S], F32, tag="cos", bufs=1)
            nc.sync.dma_start(cos_sb[:], cos_d[:])
            sin_sb = sb.tile([128, S], F32, tag="sin", bufs=1)
            nc.sync.dma_start(sin_sb[:], sin_d[:])
            r2t_sb = sb.tile([128, 128], BF16, tag="r2t", bufs=1)
            nc.sync.dma_start(r2t_sb[:], r2t_d[:])
            mask_sb = sb.tile([128, 2048], BF16, tag="mask", bufs=1)
            nc.sync.dma_start(mask_sb[:], mask_d[:])

            # v with ones column (softmax denominator):
            # [128 s, 8 pairs x 2 kvh x 2 tiles x 65] bf16
            VA_C = 65
            vaug = sb.tile([128, 8 * 2 * 2 * VA_C], BF16, tag="vaug", bufs=1)
            va = vaug[:].rearrange("p (g h t c) -> p g h t c", g=8, h=2, t=2,
                                   c=VA_C)
            nc.vector.memset(va[:, :, :, :, 64], 1.0)

            ropedq = [sb.tile([128, S], BF16, tag="ropedq", bufs=4, name=f"rq{i}")
                      for i in range(4)]
            # kv head l duplicated into both 64-row halves so QK matmul operand
            # base partitions match for q heads in either half (walrus rejects
            # tile_position with mismatched AP bases at codegen)
            kdup = [sb.tile([128, S], BF16, tag="kdup", bufs=2, name=f"kd{i}")
                    for i in range(2)]
            outt = [sb.tile([128, S], BF16, tag="outt", bufs=4, name=f"ot{i}")
                    for i in range(4)]

            # ---- weights: host-cast bf16, direct DMA (Scalar HWDGE queue),
            # interleaved with chunk-0 x tiles (GpSimd) so the first qkv
            # matmuls can start within ~1us ----
            # consumer-ordered: all x (gpsimd) + all wk (scalar) first so
            # k_piece is fully fed ~6us in; wv next (v_piece follows), wq in
            # parallel on Sync (idle after the small constants until the
            # first RS)
            xbf0 = []
            wq_sb, wk_sb, wv_sb = [], [], []
            for kt in range(KT):
                xb = sb.tile([128, 512], BF16, tag="xbf", bufs=36, name="xbf")
                nc.gpsimd.dma_start(xb[:], xt_d[128 * kt:128 * (kt + 1), 0:512])
                xbf0.append(xb)
            for kt in range(KT):
                t = sb.tile([128, 128], BF16, tag="wk", bufs=KT, name="wk")
                nc.scalar.dma_start(t[:], wk_d[128 * kt:128 * (kt + 1), :])
                wk_sb.append(t)
            for kt in range(KT):
                t = sb.tile([128, 128], BF16, tag="wv", bufs=KT, name="wv")
                nc.scalar.dma_start(t[:], wv_d[128 * kt:128 * (kt + 1), :])
                wv_sb.append(t)
            for kt in range(KT):
                t = sb.tile([128, 512], BF16, tag="wq", bufs=KT, name="wq")
                nc.sync.dma_start(t[:], wq_d[128 * kt:128 * (kt + 1), :])
                wq_sb.append(t)
            wp_sb = [sb.tile([128, DIM], BF16, tag="wp", bufs=4, name="wp")
                     for hk in range(4)]

            def load_wp():
                # emitted at the start of the first attention phase: DMA
                # overlaps attention, ready before proj(qc=0)
                for hk in range(4):
                    nc.scalar.dma_start(wp_sb[hk][:],
                                        wp_d[128 * hk:128 * (hk + 1), :])

            # quarter-width (512-col) proj outputs, one RS per quarter:
            # 16 small collectives pipelined behind compute
            ypq = [[dr.tile([512, 512], BF16, tag=f"ypq{qc}_{dc}", bufs=1,
                            name=f"ypq{qc}_{dc}") for dc in range(4)]
                   for qc in range(NQC)]
            yrsq = [[dr.tile([128, 512], BF16, tag=f"yrsq{qc}_{dc}", bufs=1,
                             name=f"yrsq{qc}_{dc}") for dc in range(4)]
                    for qc in range(NQC)]

            def rope_chunk(psum_q, ch, dest, k_mode=False):
                """dest[:, 512ch:+512] = psum_q*cos + (R2@bf16(psum_q))*sin.

                k_mode: dest is the kdup pair; head 0 -> kdup[0] rows 0:64,
                head 1 -> kdup[1] rows 64:128, other halves filled by DMA."""
                sl = slice(512 * ch, 512 * (ch + 1))
                # on Vector, not Scalar: the ACT queue is co-binding with the
                # attention exps that this rope work interleaves with
                q_sb = sb.tile([128, 512], BF16, tag="qsb", bufs=3, name="qsb")
                nc.vector.tensor_copy(q_sb[:], psum_q[:])
                prot = ps.tile([128, 512], F32, tag="mm", bufs=2, name="prot")
                nc.tensor.matmul(prot[:], r2t_sb[:], q_sb[:], start=True, stop=True)
                e1 = sb.tile([128, 512], F32, tag="e1", bufs=3, name="e1")
                nc.vector.tensor_mul(e1[:], psum_q[:], cos_sb[:, sl])
                e2 = sb.tile([128, 512], F32, tag="e2", bufs=3, name="e2")
                nc.vector.tensor_mul(e2[:], prot[:], sin_sb[:, sl])
                if not k_mode:
                    nc.vector.tensor_add(dest[:, sl], e1[:], e2[:])
                else:
                    kd0, kd1 = dest
                    nc.vector.tensor_add(kd0[0:64, sl], e1[0:64, :], e2[0:64, :])
                    nc.vector.tensor_add(kd1[64:128, sl], e1[64:128, :],
                                         e2[64:128, :])
                    nc.gpsimd.dma_start(kd0[64:128, sl], kd0[0:64, sl])
                    nc.gpsimd.dma_start(kd1[0:64, sl], kd1[64:128, sl])

            # ================= software-pipelined main loop ===================
            # Emission order interleaves three streams so every engine stays
            # dense: attention pairs for chunk qc, next chunk's qkv projection
            # (PE filler while ACT drains exps), and the previous chunk's
            # out-projection + ReduceScatter quarters. Background work lives
            # in a FIFO of generators; only the FIFO head is ever pumped so
            # mm-pool allocations from different pieces never interleave
            # (interleaving two in-flight mm users deadlocks the in-order PE
            # queue on the pool's 2-buffer rotation).

            gens = {}
            order = _deque()

            def add_gen(name, g):
                gens[name] = g
                order.append(name)

            def _pump_head():
                while order and order[0] not in gens:
                    order.popleft()
                if not order:
                    return 0
                n = order[0]
                try:
                    return next(gens[n])
                except StopIteration:
                    del gens[n]
                    order.popleft()
                    return 0

            def filler(budget_ns=900):
                spent = 0
                while order and spent < budget_ns:
                    spent += _pump_head()

            def drain_through(name):
                """FIFO-drain until `name` has completed (or was never added)."""
                while name in gens:
                    _pump_head()

            def drain_all():
                while order:
                    _pump_head()

            def g_x(ch):
                """x tiles for chunk ch - plain DMAs, emitted immediately."""
                sl = slice(512 * ch, 512 * (ch + 1))
                xs = []
                for kt in range(KT):
                    xb = sb.tile([128, 512], BF16, tag="xbf", bufs=36,
                                 name="xbf")
                    nc.gpsimd.dma_start(xb[:], xt_d[128 * kt:128 * (kt + 1), sl])
                    xs.append(xb)
                return xs

            def g_k(ch, xbf):
                pk = ps.tile([128, 512], F32, tag="mm", bufs=2, name="pk")
                for kt in range(KT):
                    nc.tensor.matmul(pk[:], wk_sb[kt][:], xbf[kt][:],
                                     start=(kt == 0), stop=(kt == KT - 1))
                    if kt % 2 == 1:
                        yield 450
                rope_chunk(pk, ch, kdup, k_mode=True)
                yield 400

            def g_v(ch, xbf):
                # all 4 S-blocks of this chunk share one mm tile (separate
                # 128-col accumulation regions)
                pv4 = ps.tile([128, 512], F32, tag="mm", bufs=2, name="pv4")
                for p in range(4):
                    st_idx = 4 * ch + p
                    pv = pv4[:, 128 * p:128 * (p + 1)]
                    for kt in range(KT):
                        nc.tensor.matmul(
                            pv[:], xbf[kt][:, 128 * p:128 * (p + 1)],
                            wv_sb[kt][:],
                            start=(kt == 0), stop=(kt == KT - 1))
                        if kt % 4 == 3:
                            yield 250
                    pvv = pv[:].rearrange("r (h c) -> r h c", h=2)
                    nc.vector.tensor_copy(
                        va[:, st_idx // 2, :, st_idx % 2, 0:64], pvv[:])

            def g_q(ch, qt, xbf):
                pq = ps.tile([128, 512], F32, tag="mm", bufs=2, name="pq")
                for kt in range(KT):
                    nc.tensor.matmul(
                        pq[:], wq_sb[kt][:, 128 * qt:128 * (qt + 1)],
                        xbf[kt][:],
                        start=(kt == 0), stop=(kt == KT - 1))
                    if kt % 2 == 1:
                        yield 450
                rope_chunk(pq, ch, ropedq[qt])
                yield 400

            def attention_pair(qc, pi, filler=None, norm_hook=None):
                """Heads (2*pi, 2*pi+1) of chunk qc, full 512-col chunk.

                QK for the even head (q rows 0:64, k at kdup rows 0:64) and
                the odd head (rows 64:128) are emitted back-to-back: walrus
                derives row_grp h0/h64 from the operand base partitions, so
                the two matmuls run CONCURRENTLY in disjoint halves of the
                PE array (row tiling) - one 512-col stream instead of two.

                filler(k): emit ~k matmuls of background work (qkv/proj).
                norm_hook: closure emitting the PREVIOUS pair's normalize;
                called after this pair's first tile so its PE work (the
                broadcast matmul) queues behind fresh QK work instead of
                head-of-line-blocking on the DVE reciprocal.
                """
                lkv = pi // 2
                qtile = ropedq[pi]
                qsl = slice(512 * qc, 512 * (qc + 1))
                po = [ps.tile([65, 512], F32, tag="av", bufs=2,
                              name=f"po{h}") for h in range(2)]
                n_tiles = 4 * (qc + 1)
                for t in range(n_tiles):
                    p = t - 4 * qc          # tile offset within the chunk
                    # causal triangle trim: tile t's scores for q cols below
                    # 128p are all-masked - skip them everywhere.
                    clo = max(0, 128 * p)
                    pscr = ps.tile([128, 1024], F32, tag="scores", bufs=2,
                                   name="pscr")
                    ps3 = pscr[:].rearrange("r (h q) -> r h q", h=2)
                    for h in range(2):
                        rows = slice(64 * h, 64 * h + 64)
                        nc.tensor.matmul(
                            ps3[:, h, clo:512],
                            kdup[lkv][rows, 128 * t:128 * (t + 1)],
                            qtile[rows, qsl.start + clo:qsl.stop],
                            start=True, stop=True)
                    expt = sb.tile([128, 1024], BF16, tag="expt", bufs=6,
                                   name="expt")
                    e3 = expt[:].rearrange("r (h q) -> r h q", h=2)
                    nc.scalar.activation(e3[:, :, clo:512], ps3[:, :, clo:512],
                                         AF.Exp, scale=SCALE)
                    if 0 <= p < 4:
                        # diagonal transition tile: mask the 128-wide block
                        lo, hi = clo, min(128 * (p + 1), 512)
                        for h in range(2):
                            reg = e3[:, h, lo:hi]
                            msk = mask_sb[:, 512 * p + lo:512 * p + hi]
                            nc.vector.tensor_mul(reg[:], reg[:], msk[:])
                    if t == 0 and norm_hook is not None:
                        # previous pair's normalize. Placement is load-bearing
                        # both ways: AFTER exp(t0) - the normalize's broadcast
                        # matmuls allocate from the scores pool, and a scores
                        # alloc before exp(t0) is emitted would hand pscr(t0)'s
                        # physical banks to the broadcast (clobbering the
                        # scores exp is about to read); BEFORE the first AV -
                        # AV(t=0) waits on the av-pool rotation, which the
                        # normalize's final multiply releases, so emitting the
                        # normalize later would deadlock the in-order PE queue.
                        norm_hook()
                    for h in range(2):
                        nc.tensor.matmul(
                            po[h][:, clo:512], va[:, t // 2, lkv, t % 2, 0:65],
                            e3[:, h, clo:512],
                            start=(t == 0), stop=(t == n_tiles - 1))
                    if qc == 0 and pi == 0 and t == 0:
                        nc.gpsimd.dma_start(dbg_ex_d[:], expt[:])
                    if filler is not None:
                        filler(900)
                if qc == 0 and pi == 0:
                    pox = sb.tile([65, 512], F32, tag="dbgpo", bufs=1,
                                  name="dbgpo")
                    nc.vector.tensor_copy(pox[:], po[0][:])
                    nc.gpsimd.dma_start(dbg_po_d[:], pox[:])

                def normalize():
                    # outt = po[0:64] * (1/po[64]) with 1/Z on the DVE
                    # (reciprocal_approx_fast, ~51 ULP); broadcast across 64
                    # partitions via ones64 matmul in the mm pool.
                    for h in range(2):
                        qrows = slice(64 * h, 64 * h + 64)
                        # stage Z to SBUF partition 0 first:
                        # reciprocal_approx_fast NaNs on partition-base-64
                        # input APs (measured; t_recip2)
                        zc = sb.tile([1, 512], F32, tag="zc", bufs=4,
                                     name="zc")
                        nc.vector.tensor_copy(zc[:], po[h][64:65, :])
                        rf = sb.tile([1, 512], F32, tag="recf", bufs=4,
                                     name="recf")
                        nc.vector.reciprocal_approx_fast(rf[:], zc[:])
                        if qc == 0 and pi == 0 and h == 0:
                            nc.gpsimd.dma_start(dbg_rf_d[:], rf[:])
                        rb = sb.tile([1, 512], BF16, tag="recb", bufs=4,
                                     name="recb")
                        nc.vector.tensor_copy(rb[:], rf[:])
                        # scores pool, NOT mm: normalize is emitted while a
                        # background qkv generator may hold an mm tile
                        # mid-accumulation; a second mm alloc here would
                        # deadlock the in-order PE queue on the 2-buf
                        # rotation. Scores allocs are always fully consumed
                        # before the next scores alloc, so this is safe.
                        prt = ps.tile([128, 1024], F32, tag="scores", bufs=2,
                                      name="prt")
                        pr = prt[0:64, 0:512]
                        nc.tensor.matmul(pr[:], ones64[:], rb[:],
                                         start=True, stop=True)
                        rbc = sb.tile([64, 512], F32, tag="rbc", bufs=3,
                                      name="rbc")
                        nc.vector.tensor_copy(rbc[:], pr[:])
                        dst = outt[pi][qrows, qsl]
                        nc.vector.tensor_mul(dst[:], po[h][0:64, :], rbc[:])
                return normalize

            def g_proj_quarter(dst_yp, dst_yrs, stiles, dc, col0=None):
                """Column quarter dc of a row-range partial projection (+ RS)."""
                if col0 is None:
                    col0 = 512 * dc
                for i, st_idx in enumerate(stiles):
                    py = ps.tile([128, 512], F32, tag="mm", bufs=2, name="py")
                    for hk in range(4):
                        nc.tensor.matmul(
                            py[:], outt[hk][:, 128 * st_idx:128 * (st_idx + 1)],
                            wp_sb[hk][:, 512 * dc:512 * (dc + 1)],
                            start=(hk == 0), stop=(hk == 3))
                    ysb = sb.tile([128, 512], BF16, tag="ysb", bufs=8, name="ysb")
                    nc.vector.tensor_copy(ysb[:], py[:])
                    nc.gpsimd.dma_start(
                        dst_yp[128 * i:128 * (i + 1), col0:col0 + 512], ysb[:])
                    yield 950
                if dst_yrs is not None:
                    nc.gpsimd.collective_compute(
                        "ReduceScatter", mybir.AluOpType.add,
                        replica_groups=groups,
                        ins=[dst_yp[:]], outs=[dst_yrs[:]])
                    yield 50

            # final chunk: one full-width RS (per-piece collective latency is
            # ~10us regardless of size, so the tail wants one big piece)
            yp3 = dr.tile([512, DIM], BF16, tag="yp3", bufs=1, name="yp3")
            yrs3 = dr.tile([128, DIM], BF16, tag="yrs3", bufs=1, name="yrs3")

            # chunk 0 qkv up front (x tiles already DMA'd above), then queue
            # the remaining q tiles as background pieces
            load_wp()
            for _ in g_k(0, xbf0):
                pass
            for _ in g_v(0, xbf0):
                pass
            for _ in g_q(0, 0, xbf0):
                pass
            for qt in range(1, 4):
                add_gen(f"q0_{qt}", g_q(0, qt, xbf0))

            pend_norm = [None]

            def norm_hook():
                if pend_norm[0] is not None:
                    pend_norm[0]()
                    pend_norm[0] = None

            for qc in range(NQC):
                if qc >= 1:
                    # proj of the previous chunk first: it is ready work and
                    # its RS pieces want to pipeline early behind compute
                    for dc in range(4):
                        add_gen(f"p{qc - 1}_{dc}",
                                g_proj_quarter(ypq[qc - 1][dc], yrsq[qc - 1][dc],
                                               [4 * (qc - 1) + p
                                                for p in range(4)],
                                               dc, col0=0))
                if qc < NQC - 1:
                    xs = g_x(qc + 1)
                    add_gen(f"k{qc + 1}", g_k(qc + 1, xs))
                    add_gen(f"v{qc + 1}", g_v(qc + 1, xs))
                    for qt in range(4):
                        add_gen(f"q{qc + 1}_{qt}", g_q(qc + 1, qt, xs))
                for pi in range(4):
                    drain_through(f"q{qc}_{pi}")
                    norm = attention_pair(qc, pi, filler=filler,
                                          norm_hook=norm_hook)
                    norm()  # DEBUG: inline normalize (no deferral)
            drain_all()
            for dc in range(4):
                for _ in g_proj_quarter(yp3, None, [12, 13, 14, 15], dc):
                    pass
            nc.gpsimd.collective_compute(
                "ReduceScatter", mybir.AluOpType.add,
                replica_groups=groups, ins=[yp3[:]], outs=[yrs3[:]])

            # output copies last: every RS has fired; nothing queues behind them
            for qc in range(NQC - 1):
                for dc in range(4):
                    nc.gpsimd.dma_start(
                        y_d[128 * qc:128 * (qc + 1), 512 * dc:512 * (dc + 1)],
                        yrsq[qc][dc][:])
            nc.gpsimd.dma_start(y_d[384:512, :], yrs3[:])
            nc.gpsimd.dma_start(dbg_out_d[:], outt[0][:])

    nc.compile()
    _CACHED_NC = nc
    return nc


def _consts():
    half = HD // 2
    inv_freq = 1.0 / (THETA ** (np.arange(half, dtype=np.float32) * 2.0 / HD))
    ang = np.arange(S, dtype=np.float32)[:, None] * inv_freq      # [S, 32]
    cos = np.cos(ang).T.astype(np.float32)                        # [32, S]
    sin = np.sin(ang).T.astype(np.float32)
    cos64 = np.concatenate([cos, cos], 0)
    sin64 = np.concatenate([sin, sin], 0)
    cosT = np.concatenate([cos64, cos64], 0)                      # [128, S]
    sinT = np.concatenate([sin64, sin64], 0)

    M = np.zeros((HD, HD), np.float32)
    for i in range(half):
        M[i, i + half] = -1.0
        M[i + half, i] = 1.0
    M2 = np.zeros((128, 128), np.float32)
    M2[:64, :64] = M
    M2[64:, 64:] = M
    r2t = M2.T.astype(ml_dtypes.bfloat16)

    masks = np.zeros((128, 2048), np.float32)
    q_idx = np.arange(512)[None, :]
    for p in range(4):
        kv_idx = np.arange(128)[:, None] + 128 * p
        masks[:, 512 * p:512 * (p + 1)] = (q_idx >= kv_idx)
    maskt = masks.astype(ml_dtypes.bfloat16)
    return cosT, sinT, r2t, maskt


def _in_maps(x, w_qkv, w_proj):
    cosT, sinT, r2t, maskt = _consts()
    bf = ml_dtypes.bfloat16
    maps = []
    for c in range(NCORES):
        b, g = c // 4, c % 4
        maps.append({
            "xt": np.ascontiguousarray(x[b].T).astype(bf),
            "wq": np.ascontiguousarray(
                w_qkv[:, 512 * g:512 * (g + 1)]).astype(bf),
            "wk": np.ascontiguousarray(
                w_qkv[:, 2048 + 128 * g:2048 + 128 * (g + 1)]).astype(bf),
            "wv": np.ascontiguousarray(
                w_qkv[:, 2560 + 128 * g:2560 + 128 * (g + 1)]).astype(bf),
            "wp": np.ascontiguousarray(
                w_proj[512 * g:512 * (g + 1), :]).astype(bf),
            "cost": cosT, "sint": sinT, "r2t": r2t, "maskt": maskt,
        })
    return maps


def _assemble(results, b_proj):
    out = np.zeros((B, S, DIM), np.float32)
    for c in range(NCORES):
        b, j = c // 4, c % 4
        y = results[c]["y"]                    # [512, DIM]
        for qc in range(NQC):
            rows = slice(512 * qc + 128 * j, 512 * qc + 128 * (j + 1))
            out[b, rows, :] = y[128 * qc:128 * (qc + 1), :]
    out += b_proj[None, None, :].astype(np.float32)
    return out


def run(x, w_qkv, w_proj, b_proj, trace=False):
    nc = build_nc()
    res = run_bass_kernel_spmd(nc, _in_maps(x, w_qkv, w_proj),
                               core_ids=list(range(NCORES)), trace=trace)
    return _assemble(res.results, np.asarray(b_proj)), res


def kernel(x, w_qkv, w_proj, b_proj):
    x = np.asarray(x)
    w_qkv = np.asarray(w_qkv)
    w_proj = np.asarray(w_proj)
    b_proj = np.asarray(b_proj)
    out, _ = run(x, w_qkv, w_proj, b_proj, trace=False)
    return out

